# revision 34
# baseline (speedup 1.0000x reference)
"""SSD-style multibox loss (Huber loc + softmax conf with hard-negative
mining) on 8 Trainium2 NeuronCores, pure data-parallel over the batch.

Full inputs [32, 8732, ...] come in; each core processes 4 batch rows.
The host pads every per-core shard from 8732 to 8736 anchors so the
on-chip layout is an exact 32-partition x 273-group grid per batch row
(4 row-blocks x 32 partitions = 128 partitions).  Pad anchors are made
*positives* with zero Huber loss (both bbox tensors padded with 1.0,
labels/logits padded with 0.0): they never enter hard-negative mining,
contribute exactly ln(81) each to the positive-conf sum and 4 per row to
the positive count, both corrected exactly in the host combine (and on
device for k = 3*pos).  This removes every ragged-tail / pad-fill DMA.

DMA schedule (measured on this part under 8-core load): a SINGLE ring
with every transfer spanning all 128 partitions sustains the best rate
(SWDGE chunked ~330 GB/s/core); two concurrent rings drop to ~220
(packet-granular SDMA ring round-robin destroys HBM locality) and
32-partition row-block DMAs to ~185 (partitions 0-63 map to only half
the SDMA engines).  Casting during DMA costs ~13% line rate, so the
stream stays f32.  ALL transfers ride the gpsimd (SWDGE) ring in strict
priority order:
  bbox (2 x 0.56MB) -> pred (7 x 39-group chunks, 1.62MB) -> labels
  (six 39-group chunks + 20/19 tail so the post-stream dot is half
  size; 5-deep buffer ring) -> per-partition partials out.
The gpsimd queue carries ONLY DMAs (its tensor_scalar ucode is ~6x
slower than DVE and SWDGE descriptor gen competes with any compute).

Device computes, per core, into a [128, NF] partials tile:
  - sumexp / lse per anchor (ACT exp -> bf16, DVE reduce over 81 classes)
  - pos mask / per-partition pos count (from actual_bbox_deltas)
  - Huber localization sum over positives, via
    huber(a) = 0.5*m*(2a - m), m = min(a, 1)
  - S1 = sum(actual_labels * pred_labels)   (per-chunk DVE dot-accums,
    chasing the label stream; S4 = sum_all pred0 runs on ACT)
  - S2 = sum_pos lse, S3 = sum_pos pred0
  - hard-negative mining threshold t_r per batch row (k_r = 3*(pos_r-4))
    by an 8-step vectorized binary search on ACT+PE, hidden under the
    label stream; the entire mining tail is ONE ACT relu-accumulate,
    since sum_topk = sum(v*[v>t]) + t*(k - cnt) = sum(relu(v-t)) + t*k.
The host sums the 8 cores' [128, NF] partials (float64) and finishes:
  neg = relu_sum + t*k per row,
  conf = lse - dot(labels, pred) with dot = pred0 for negatives, so
  sum_pos conf = S2c - (S1 - (S4 - S3)).
"""

import numpy as np

import concourse.bass as bass
import concourse.bacc as bacc
import concourse.tile as tile
import concourse.mybir as mybir
from concourse.bass_utils import run_bass_kernel_spmd

F32 = mybir.dt.float32
BF16 = mybir.dt.bfloat16
AX = mybir.AxisListType
OP = mybir.AluOpType
AF = mybir.ActivationFunctionType

B, P, C = 32, 8732, 81
NCORES = 8
BL = B // NCORES            # batch rows per core = 4
PPR = 32                    # partitions per row-block
G = 273                     # anchor groups per partition
PP = PPR * G                # padded anchors per row = 8736
NPAD = PP - P               # pad anchors per row = 4
NEG_BIG = -1.0e30
NITER = 8                  # binary-search iterations (range [0, 32))
T0 = 16.0
NF = 16                     # output partial columns

CH = 39                     # pred chunk (7 per row, 1.62MB each)
NCH = G // CH
# label chunks: six 39-group chunks, then 20+19 so the tail dot after the
# last byte lands is half-size
LBL_CH = [(39 * k, 39 * (k + 1)) for k in range(6)] + [(234, 254), (254, 273)]
LN81 = float(np.log(81.0))

# column map of the [128, NF] per-core partials
COL_LOC, COL_S2, COL_S3, COL_S4, COL_POS = 0, 1, 2, 3, 4
COL_RELU, COL_KCOL, COL_TCOL = 5, 6, 7
COL_S1 = 8                  # .. COL_S1 + NCH - 1


def _ap4(dram, inner, g0, g1):
    """4D source AP over [BL, PP, inner] DRAM covering groups [g0, g1) of
    every partition: dst partition 32*r+q holds groups [q*G+g0, q*G+g1)
    of batch row r.  Spans all 128 partitions in ONE dma_start (128
    contiguous per-partition runs -> full 16-engine fan-out)."""
    return bass.AP(dram, g0 * inner,
                   [[PP * inner, BL], [G * inner, PPR],
                    [inner, g1 - g0], [1, inner]])


def build():
    nc = bacc.Bacc("TRN2", target_bir_lowering=False, debug=False)

    d_ab = nc.dram_tensor("actual_bbox_deltas", [BL, PP, 4], F32, kind="ExternalInput")
    d_al = nc.dram_tensor("actual_labels", [BL, PP, C], F32, kind="ExternalInput")
    d_pb = nc.dram_tensor("pred_bbox_deltas", [BL, PP, 4], F32, kind="ExternalInput")
    d_pl = nc.dram_tensor("pred_labels", [BL, PP, C], F32, kind="ExternalInput")
    d_out = nc.dram_tensor("out", [128, NF], F32, kind="ExternalOutput")

    with tile.TileContext(nc) as tc:
        with (
            tc.tile_pool(name="const", bufs=1) as constp,
            tc.tile_pool(name="resident", bufs=1) as resp,
            tc.tile_pool(name="bbox", bufs=1) as bbp,
            tc.tile_pool(name="hub", bufs=1) as hubp,
            tc.tile_pool(name="expj", bufs=2) as expp,
            tc.tile_pool(name="lblchunk", bufs=5) as lblp,
            tc.tile_pool(name="dotj", bufs=1) as djp,
            tc.tile_pool(name="small", bufs=1) as smallp,
            tc.tile_pool(name="mine", bufs=2) as minep,
            tc.tile_pool(name="psum", bufs=2, space="PSUM") as psump,
        ):
            # ---- the whole input stream, in priority order, one ring ----
            abt = bbp.tile([128, G, 4], F32, tag="abt")
            pbt = bbp.tile([128, G, 4], F32, tag="pbt")
            nc.gpsimd.dma_start(abt[:, :, :], _ap4(d_ab, 4, 0, G))
            nc.gpsimd.dma_start(pbt[:, :, :], _ap4(d_pb, 4, 0, G))

            pred = resp.tile([128, G, C], F32, tag="pred")
            lbls = [None] * len(LBL_CH)

            def pred_dma(k):
                nc.gpsimd.dma_start(pred[:, k * CH:(k + 1) * CH, :],
                                    _ap4(d_pl, C, k * CH, (k + 1) * CH))

            def lbl_dma(k):
                g0, g1 = LBL_CH[k]
                lbl = lblp.tile([128, g1 - g0, C], F32, tag="lbl")
                lbls[k] = lbl
                nc.gpsimd.dma_start(lbl[:, :, :], _ap4(d_al, C, g0, g1))

            for k in range(NCH):
                pred_dma(k)
            for k in range(len(LBL_CH)):
                lbl_dma(k)

            # ---- constants (DVE memsets; the gpsimd queue is pure DMA) ----
            blockones = constp.tile([128, 128], F32)
            nc.vector.memset(blockones[:, :], 0.0)
            for r in range(BL):
                nc.vector.memset(
                    blockones[r * PPR:(r + 1) * PPR, r * PPR:(r + 1) * PPR], 1.0)
            fpart = constp.tile([128, NF], F32)
            nc.vector.memset(fpart[:, :], 0.0)
            negt0 = minep.tile([128, 1], F32, tag="negt")
            nc.vector.memset(negt0[:, :], -T0)

            # ---- bbox compute ----
            absmax = bbp.tile([128, G], F32, tag="absmax")
            nc.vector.tensor_reduce(absmax[:, :], abt[:, :, :], AX.X, OP.max,
                                    apply_absolute_value=True)
            posmask = bbp.tile([128, G], F32, tag="posmask")
            nc.vector.tensor_scalar(posmask[:, :], absmax[:, :], 0.0, None, OP.is_gt)

            pospart = bbp.tile([128, 1], F32, tag="pospart")
            nc.vector.tensor_reduce(pospart[:, :], posmask[:, :], AX.X, OP.add)
            nc.vector.tensor_copy(fpart[:, COL_POS:COL_POS + 1], pospart[:, :])
            pos_rep = psump.tile([128, 1], F32, tag="posrep")
            nc.tensor.matmul(pos_rep[:, :], blockones[:, :], pospart[:, :])
            # k = 3*(pos - NPAD);  sign(cnt-k) = sign(srep + negk2) with
            # srep = 2*cnt - PP  ->  negk2 = PP - 2k = PP + 6*NPAD - 6*pos
            kcol = bbp.tile([128, 1], F32, tag="kcol")
            nc.vector.tensor_scalar(kcol[:, :], pos_rep[:, :], 3.0,
                                    -3.0 * NPAD, OP.mult, OP.add)
            nc.vector.tensor_copy(fpart[:, COL_KCOL:COL_KCOL + 1], kcol[:, :])
            negk2 = bbp.tile([128, 1], F32, tag="negk2")
            nc.vector.tensor_scalar(negk2[:, :], pos_rep[:, :], -6.0,
                                    float(PP + 6 * NPAD), OP.mult, OP.add)

            # Huber loc loss via huber(a) = 0.5*m*(2a - m), m = min(a, 1):
            # a<=1 -> 0.5a^2 ; a>1 -> a - 0.5.  One ACT op, rest DVE.
            dt_ = hubp.tile([128, G, 4], F32, tag="hd")
            nc.vector.tensor_sub(dt_[:, :, :], pbt[:, :, :], abt[:, :, :])
            nc.scalar.activation(dt_[:, :, :], dt_[:, :, :], AF.Abs)  # a = |d|
            mt = hubp.tile([128, G, 4], F32, tag="hm")
            nc.vector.tensor_single_scalar(mt[:, :, :], dt_[:, :, :], 1.0, OP.min)
            st = hubp.tile([128, G, 4], F32, tag="hs")
            nc.vector.scalar_tensor_tensor(                       # w = 2a - m
                st[:, :, :], dt_[:, :, :], 2.0, mt[:, :, :],
                OP.mult, OP.subtract)
            nc.vector.tensor_mul(st[:, :, :], st[:, :, :], mt[:, :, :])
            hpb = hubp.tile([128, G], F32, tag="hpb")
            nc.vector.tensor_reduce(hpb[:, :], st[:, :, :], AX.X, OP.add)
            hjunk = hubp.tile([128, G], F32, tag="hjunk")
            nc.vector.scalar_tensor_tensor(
                hjunk[:, :], hpb[:, :], 0.125, posmask[:, :], OP.mult, OP.mult,
                accum_out=fpart[:, COL_LOC:COL_LOC + 1])

            # ---- exp (-> bf16) + per-anchor sumexp over pred ----
            sumexp = resp.tile([128, G], F32, tag="sumexp")
            for k in range(NCH):
                sl = pred[:, k * CH:(k + 1) * CH, :]
                ex = expp.tile([128, CH, C], BF16, tag="exp")
                nc.scalar.activation(ex[:, :, :], sl, AF.Exp)
                nc.vector.tensor_reduce(sumexp[:, k * CH:(k + 1) * CH],
                                        ex[:, :, :], AX.X, OP.add)

            lse = resp.tile([128, G], F32, tag="lse")
            nc.scalar.activation(lse[:, :], sumexp[:, :], AF.Ln)
            pred0 = pred[:, :, 0]
            nconf = resp.tile([128, G], F32, tag="nconf")
            nc.vector.tensor_sub(nconf[:, :], lse[:, :], pred0)
            masked = resp.tile([128, G], F32, tag="masked")
            i_masked = nc.vector.scalar_tensor_tensor(
                masked[:, :], posmask[:, :], NEG_BIG, nconf[:, :], OP.mult, OP.add)

            # S2, S3, S4
            j2 = smallp.tile([128, G], F32, tag="sjunk")
            nc.vector.scalar_tensor_tensor(
                j2[:, :], posmask[:, :], 0.0, lse[:, :], OP.bypass, OP.mult,
                accum_out=fpart[:, COL_S2:COL_S2 + 1])
            j3 = smallp.tile([128, G], F32, tag="sjunk")
            nc.vector.scalar_tensor_tensor(
                j3[:, :], posmask[:, :], 0.0, pred0, OP.bypass, OP.mult,
                accum_out=fpart[:, COL_S3:COL_S3 + 1])

            # ---- hard-negative mining: binary search on t per row (ACT+PE
            # only, hidden under the label stream) ----
            negt = negt0
            for i in range(NITER):
                cjunk = minep.tile([128, G], F32, tag="cjunk")
                cnt = minep.tile([128, 1], F32, tag="cnt")
                # sum(sign(masked - t)) = cnt_gt - cnt_le   (per partition)
                nc.scalar.activation(cjunk[:, :], masked[:, :], AF.Sign,
                                     bias=negt[:, :], accum_out=cnt[:, :])
                srep = psump.tile([128, 1], F32, tag="srep")
                nc.tensor.matmul(srep[:, :], blockones[:, :], cnt[:, :])
                # s = sign(sum_rep + negk2) : +1 -> count>k -> t too low
                sdir = minep.tile([128, 1], F32, tag="sdir")
                nc.scalar.activation(sdir[:, :], srep[:, :], AF.Sign,
                                     bias=negk2[:, :])
                delta = T0 / (2 ** (i + 1))
                negt2 = minep.tile([128, 1], F32, tag="negt")
                nc.scalar.activation(negt2[:, :], sdir[:, :], AF.Identity,
                                     bias=negt[:, :], scale=-delta)
                negt = negt2

            # ---- label dots (DVE, in label-arrival order).  Dots 0-1 run
            # as soon as their chunks land; `masked` (which gates the whole
            # mining chain) is pinned between dots 1 and 2 so neither the
            # mining start nor the dot pipeline slips. ----
            dj = djp.tile([128, CH, C], BF16, tag="dotjunk")
            dj2 = djp.tile([128, CH, C], BF16, tag="dotjunk2")
            i_dot = None
            for k, (g0, g1) in enumerate(LBL_CH):
                if k == 0:
                    # product on GpSimd, sum via ACT identity-accumulate in
                    # the ACT gap before mining: takes one dot off the
                    # saturated DVE chain at zero DVE cost
                    nc.gpsimd.tensor_mul(dj2[:, :, :], lbls[0][:, :, :],
                                         pred[:, g0:g1, :])
                    nc.scalar.activation(
                        dj2[:, :, :], dj2[:, :, :], AF.Identity,
                        accum_out=fpart[:, COL_S1:COL_S1 + 1])
                    continue
                i_dot = nc.vector.scalar_tensor_tensor(
                    dj[:, 0:g1 - g0, :], lbls[k][:, :, :], 0.0,
                    pred[:, g0:g1, :], OP.bypass, OP.mult,
                    accum_out=fpart[:, COL_S1 + k:COL_S1 + k + 1])
                if k == 1:
                    tile.add_dep_helper(i_masked.ins, i_dot.ins,
                                        reason="masked after dot1")
                if k == 2:
                    tile.add_dep_helper(i_dot.ins, i_masked.ins,
                                        reason="dot2 after masked")

            # final mining pass, all on ACT:  sum_topk = sum(v*[v>t])
            # + t*(k - cnt) = sum(relu(v - t)) + t*k  exactly, so one
            # relu-accumulate is the entire tail (host adds t*k).
            nc.scalar.activation(fpart[:, COL_TCOL:COL_TCOL + 1], negt[:, :],
                                 AF.Identity, scale=-1.0)
            j4 = smallp.tile([128, G], F32, tag="sjunk")
            nc.scalar.activation(j4[:, :], pred0, AF.Identity,
                                 accum_out=fpart[:, COL_S4:COL_S4 + 1])
            rj = minep.tile([128, G], F32, tag="cjunk")
            nc.scalar.activation(rj[:, :], masked[:, :], AF.Relu,
                                 bias=negt[:, :],
                                 accum_out=fpart[:, COL_RELU:COL_RELU + 1])

            # ---- per-partition partials out; host does the final combine ----
            nc.sync.dma_start(d_out[:, :], fpart[:, :])

    nc.compile()
    return nc


_nc = None


def _pad_shard(src, inner, fill):
    """[BL, P, inner] -> [BL, PP, inner] with constant-filled pad anchors."""
    out = np.empty((BL, PP, inner), np.float32)
    out[:, :P] = src
    out[:, P:] = fill
    return out


def kernel(actual_bbox_deltas, actual_labels, pred_bbox_deltas, pred_labels):
    global _nc
    if _nc is None:
        _nc = build()

    in_maps = []
    for core in range(NCORES):
        r0 = core * BL
        in_maps.append({
            "actual_bbox_deltas": _pad_shard(
                actual_bbox_deltas[r0:r0 + BL], 4, 1.0),
            "actual_labels": _pad_shard(actual_labels[r0:r0 + BL], C, 0.0),
            "pred_bbox_deltas": _pad_shard(
                pred_bbox_deltas[r0:r0 + BL], 4, 1.0),
            "pred_labels": _pad_shard(pred_labels[r0:r0 + BL], C, 0.0),
        })

    res = run_bass_kernel_spmd(_nc, in_maps, core_ids=list(range(NCORES)))
    loc = conf = pos = 0.0
    npad_core = BL * NPAD
    for core in range(NCORES):
        o = res.results[core]["out"].astype(np.float64)
        s1 = o[:, COL_S1:COL_S1 + len(LBL_CH)].sum()
        loc += o[:, COL_LOC].sum()
        neg = (o[:, COL_RELU]
               + o[:, COL_TCOL] * o[:, COL_KCOL] / PPR).sum()
        conf += (o[:, COL_S2].sum() - npad_core * LN81) - s1 \
            + o[:, COL_S4].sum() - o[:, COL_S3].sum() + neg
        pos += o[:, COL_POS].sum() - npad_core
    if pos == 0:
        return (np.float32(0.0), np.float32(0.0))
    return (np.float32(loc / pos), np.float32(conf / pos))


# revision 35
# speedup vs baseline: 1.0397x; 1.0397x over previous
"""SSD-style multibox loss (Huber loc + softmax conf with hard-negative
mining) on 8 Trainium2 NeuronCores, pure data-parallel over the batch.

Full inputs [32, 8732, ...] come in; each core processes 4 batch rows.
The host pads every per-core shard from 8732 to 8736 anchors so the
on-chip layout is an exact 32-partition x 273-group grid per batch row
(4 row-blocks x 32 partitions = 128 partitions).  Pad anchors are made
*positives* with zero Huber loss (both bbox tensors padded with 1.0,
labels/logits padded with 0.0): they never enter hard-negative mining,
contribute exactly ln(81) each to the positive-conf sum and 4 per row to
the positive count, both corrected exactly in the host combine (and on
device for k = 3*pos).  This removes every ragged-tail / pad-fill DMA.

DMA schedule (measured on this part under 8-core load): a SINGLE ring
with every transfer spanning all 128 partitions sustains the best rate
(SWDGE chunked ~330 GB/s/core); two concurrent rings drop to ~220
(packet-granular SDMA ring round-robin destroys HBM locality) and
32-partition row-block DMAs to ~185 (partitions 0-63 map to only half
the SDMA engines).  Casting during DMA costs ~13% line rate, so the
stream stays f32.  ALL transfers ride the gpsimd (SWDGE) ring in strict
priority order:
  bbox (2 x 0.56MB) -> pred (7 x 39-group chunks, 1.62MB) -> labels
  (six 39-group chunks + 20/19 tail so the post-stream dot is half
  size; 5-deep buffer ring) -> per-partition partials out.
The gpsimd queue carries ONLY DMAs (its tensor_scalar ucode is ~6x
slower than DVE and SWDGE descriptor gen competes with any compute).

Device computes, per core, into a [128, NF] partials tile:
  - sumexp / lse per anchor (ACT exp -> bf16, DVE reduce over 81 classes)
  - pos mask / per-partition pos count (from actual_bbox_deltas)
  - Huber localization sum over positives, via
    huber(a) = 0.5*m*(2a - m), m = min(a, 1)
  - S1 = sum(actual_labels * pred_labels)   (per-chunk DVE dot-accums,
    chasing the label stream; S4 = sum_all pred0 runs on ACT)
  - S2 = sum_pos lse, S3 = sum_pos pred0
  - hard-negative mining threshold t_r per batch row (k_r = 3*(pos_r-4))
    by an 8-step vectorized binary search on ACT+PE, hidden under the
    label stream; the entire mining tail is ONE ACT relu-accumulate,
    since sum_topk = sum(v*[v>t]) + t*(k - cnt) = sum(relu(v-t)) + t*k.
The host sums the 8 cores' [128, NF] partials (float64) and finishes:
  neg = relu_sum + t*k per row,
  conf = lse - dot(labels, pred) with dot = pred0 for negatives, so
  sum_pos conf = S2c - (S1 - (S4 - S3)).
"""

import numpy as np

import concourse.bass as bass
import concourse.bacc as bacc
import concourse.tile as tile
import concourse.mybir as mybir
from concourse.bass_utils import run_bass_kernel_spmd

F32 = mybir.dt.float32
BF16 = mybir.dt.bfloat16
AX = mybir.AxisListType
OP = mybir.AluOpType
AF = mybir.ActivationFunctionType

B, P, C = 32, 8732, 81
NCORES = 8
BL = B // NCORES            # batch rows per core = 4
PPR = 32                    # partitions per row-block
G = 273                     # anchor groups per partition
PP = PPR * G                # padded anchors per row = 8736
NPAD = PP - P               # pad anchors per row = 4
NEG_BIG = -1.0e30
NITER = 8                  # binary-search iterations (range [0, 32))
T0 = 16.0
NF = 16                     # output partial columns

CH = 39                     # pred chunk (7 per row, 1.62MB each)
NCH = G // CH
# label chunks: six 39-group chunks, then 20+19 so the tail dot after the
# last byte lands is half-size
LBL_CH = [(39 * k, 39 * (k + 1)) for k in range(6)] + [(234, 254), (254, 273)]
LN81 = float(np.log(81.0))

# column map of the [128, NF] per-core partials
COL_LOC, COL_S2, COL_S3, COL_S4, COL_POS = 0, 1, 2, 3, 4
COL_RELU, COL_KCOL, COL_TCOL = 5, 6, 7
COL_S1 = 8                  # .. COL_S1 + NCH - 1


def _ap4(dram, inner, g0, g1):
    """4D source AP over [BL, PP, inner] DRAM covering groups [g0, g1) of
    every partition: dst partition 32*r+q holds groups [q*G+g0, q*G+g1)
    of batch row r.  Spans all 128 partitions in ONE dma_start (128
    contiguous per-partition runs -> full 16-engine fan-out)."""
    return bass.AP(dram, g0 * inner,
                   [[PP * inner, BL], [G * inner, PPR],
                    [inner, g1 - g0], [1, inner]])


def build():
    nc = bacc.Bacc("TRN2", target_bir_lowering=False, debug=False)

    d_ab = nc.dram_tensor("actual_bbox_deltas", [BL, PP, 4], F32, kind="ExternalInput")
    d_al = nc.dram_tensor("actual_labels", [BL, PP, C], F32, kind="ExternalInput")
    d_pb = nc.dram_tensor("pred_bbox_deltas", [BL, PP, 4], F32, kind="ExternalInput")
    d_pl = nc.dram_tensor("pred_labels", [BL, PP, C], F32, kind="ExternalInput")
    d_out = nc.dram_tensor("out", [128, NF], F32, kind="ExternalOutput")

    with tile.TileContext(nc) as tc:
        with (
            tc.tile_pool(name="const", bufs=1) as constp,
            tc.tile_pool(name="resident", bufs=1) as resp,
            tc.tile_pool(name="bbox", bufs=1) as bbp,
            tc.tile_pool(name="hub", bufs=1) as hubp,
            tc.tile_pool(name="expj", bufs=2) as expp,
            tc.tile_pool(name="lblchunk", bufs=5) as lblp,
            tc.tile_pool(name="dotj", bufs=1) as djp,
            tc.tile_pool(name="small", bufs=2) as smallp,
            tc.tile_pool(name="mine", bufs=2) as minep,
            tc.tile_pool(name="psum", bufs=2, space="PSUM") as psump,
        ):
            # ---- the whole input stream, in priority order, one ring ----
            abt = bbp.tile([128, G, 4], F32, tag="abt")
            pbt = bbp.tile([128, G, 4], F32, tag="pbt")
            nc.gpsimd.dma_start(abt[:, :, :], _ap4(d_ab, 4, 0, G))
            nc.gpsimd.dma_start(pbt[:, :, :], _ap4(d_pb, 4, 0, G))

            pred = resp.tile([128, G, C], F32, tag="pred")
            lbls = [None] * len(LBL_CH)

            def pred_dma(k):
                nc.gpsimd.dma_start(pred[:, k * CH:(k + 1) * CH, :],
                                    _ap4(d_pl, C, k * CH, (k + 1) * CH))

            def lbl_dma(k):
                g0, g1 = LBL_CH[k]
                lbl = lblp.tile([128, g1 - g0, C], F32, tag="lbl")
                lbls[k] = lbl
                nc.gpsimd.dma_start(lbl[:, :, :], _ap4(d_al, C, g0, g1))

            for k in range(NCH):
                pred_dma(k)
            for k in range(len(LBL_CH)):
                lbl_dma(k)

            # ---- constants (DVE memsets; the gpsimd queue is pure DMA) ----
            blockones = constp.tile([128, 128], F32)
            nc.vector.memset(blockones[:, :], 0.0)
            for r in range(BL):
                nc.vector.memset(
                    blockones[r * PPR:(r + 1) * PPR, r * PPR:(r + 1) * PPR], 1.0)
            fpart = constp.tile([128, NF], F32)
            nc.vector.memset(fpart[:, :], 0.0)
            negt0 = minep.tile([128, 1], F32, tag="negt")
            nc.vector.memset(negt0[:, :], -T0)

            # ---- bbox compute ----
            absmax = bbp.tile([128, G], F32, tag="absmax")
            nc.vector.tensor_reduce(absmax[:, :], abt[:, :, :], AX.X, OP.max,
                                    apply_absolute_value=True)
            posmask = bbp.tile([128, G], F32, tag="posmask")
            nc.vector.tensor_scalar(posmask[:, :], absmax[:, :], 0.0, None, OP.is_gt)

            pospart = bbp.tile([128, 1], F32, tag="pospart")
            nc.vector.tensor_reduce(pospart[:, :], posmask[:, :], AX.X, OP.add)
            nc.vector.tensor_copy(fpart[:, COL_POS:COL_POS + 1], pospart[:, :])
            pos_rep = psump.tile([128, 1], F32, tag="posrep")
            nc.tensor.matmul(pos_rep[:, :], blockones[:, :], pospart[:, :])
            # k = 3*(pos - NPAD);  sign(cnt-k) = sign(srep + negk2) with
            # srep = 2*cnt - PP  ->  negk2 = PP - 2k = PP + 6*NPAD - 6*pos
            kcol = bbp.tile([128, 1], F32, tag="kcol")
            nc.vector.tensor_scalar(kcol[:, :], pos_rep[:, :], 3.0,
                                    -3.0 * NPAD, OP.mult, OP.add)
            nc.vector.tensor_copy(fpart[:, COL_KCOL:COL_KCOL + 1], kcol[:, :])
            negk2 = bbp.tile([128, 1], F32, tag="negk2")
            nc.vector.tensor_scalar(negk2[:, :], pos_rep[:, :], -6.0,
                                    float(PP + 6 * NPAD), OP.mult, OP.add)

            # Huber loc loss via huber(a) = 0.5*m*(2a - m), m = min(a, 1):
            # a<=1 -> 0.5a^2 ; a>1 -> a - 0.5.  One ACT op, rest DVE.
            dt_ = hubp.tile([128, G, 4], F32, tag="hd")
            nc.vector.tensor_sub(dt_[:, :, :], pbt[:, :, :], abt[:, :, :])
            nc.scalar.activation(dt_[:, :, :], dt_[:, :, :], AF.Abs)  # a = |d|
            mt = hubp.tile([128, G, 4], F32, tag="hm")
            nc.vector.tensor_single_scalar(mt[:, :, :], dt_[:, :, :], 1.0, OP.min)
            st = hubp.tile([128, G, 4], F32, tag="hs")
            nc.vector.scalar_tensor_tensor(                       # w = 2a - m
                st[:, :, :], dt_[:, :, :], 2.0, mt[:, :, :],
                OP.mult, OP.subtract)
            nc.vector.tensor_mul(st[:, :, :], st[:, :, :], mt[:, :, :])
            hpb = hubp.tile([128, G], F32, tag="hpb")
            nc.vector.tensor_reduce(hpb[:, :], st[:, :, :], AX.X, OP.add)
            hjunk = hubp.tile([128, G], F32, tag="hjunk")
            nc.vector.scalar_tensor_tensor(
                hjunk[:, :], hpb[:, :], 0.125, posmask[:, :], OP.mult, OP.mult,
                accum_out=fpart[:, COL_LOC:COL_LOC + 1])

            # ---- exp (-> bf16) + per-anchor sumexp over pred ----
            sumexp = resp.tile([128, G], F32, tag="sumexp")
            for k in range(NCH):
                sl = pred[:, k * CH:(k + 1) * CH, :]
                ex = expp.tile([128, CH, C], BF16, tag="exp")
                nc.scalar.activation(ex[:, :, :], sl, AF.Exp)
                nc.vector.tensor_reduce(sumexp[:, k * CH:(k + 1) * CH],
                                        ex[:, :, :], AX.X, OP.add)

            lse = resp.tile([128, G], F32, tag="lse")
            nc.scalar.activation(lse[:, :], sumexp[:, :], AF.Ln)
            pred0 = pred[:, :, 0]
            nconf = resp.tile([128, G], F32, tag="nconf")
            nc.vector.tensor_sub(nconf[:, :], lse[:, :], pred0)
            masked = resp.tile([128, G], F32, tag="masked")
            i_masked = nc.vector.scalar_tensor_tensor(
                masked[:, :], posmask[:, :], NEG_BIG, nconf[:, :], OP.mult, OP.add)

            # S2, S3, S4
            j2 = smallp.tile([128, G], F32, tag="sjunk")
            nc.vector.scalar_tensor_tensor(
                j2[:, :], posmask[:, :], 0.0, lse[:, :], OP.bypass, OP.mult,
                accum_out=fpart[:, COL_S2:COL_S2 + 1])
            j3 = smallp.tile([128, G], F32, tag="sjunk")
            nc.vector.scalar_tensor_tensor(
                j3[:, :], posmask[:, :], 0.0, pred0, OP.bypass, OP.mult,
                accum_out=fpart[:, COL_S3:COL_S3 + 1])

            # ---- hard-negative mining: binary search on t per row (ACT+PE
            # only, hidden under the label stream) ----
            negt = negt0
            for i in range(NITER):
                cjunk = minep.tile([128, G], F32, tag="cjunk")
                cnt = minep.tile([128, 1], F32, tag="cnt")
                # sum(sign(masked - t)) = cnt_gt - cnt_le   (per partition)
                nc.scalar.activation(cjunk[:, :], masked[:, :], AF.Sign,
                                     bias=negt[:, :], accum_out=cnt[:, :])
                srep = psump.tile([128, 1], F32, tag="srep")
                nc.tensor.matmul(srep[:, :], blockones[:, :], cnt[:, :])
                # s = sign(sum_rep + negk2) : +1 -> count>k -> t too low
                sdir = minep.tile([128, 1], F32, tag="sdir")
                nc.scalar.activation(sdir[:, :], srep[:, :], AF.Sign,
                                     bias=negk2[:, :])
                delta = T0 / (2 ** (i + 1))
                negt2 = minep.tile([128, 1], F32, tag="negt")
                nc.scalar.activation(negt2[:, :], sdir[:, :], AF.Identity,
                                     bias=negt[:, :], scale=-delta)
                negt = negt2

            # ---- label dots (DVE, in label-arrival order).  Dots 0-1 run
            # as soon as their chunks land; `masked` (which gates the whole
            # mining chain) is pinned between dots 1 and 2 so neither the
            # mining start nor the dot pipeline slips. ----
            dj = djp.tile([128, CH, C], BF16, tag="dotjunk")
            i_dot = None
            for k, (g0, g1) in enumerate(LBL_CH):
                i_dot = nc.vector.scalar_tensor_tensor(
                    dj[:, 0:g1 - g0, :], lbls[k][:, :, :], 0.0,
                    pred[:, g0:g1, :], OP.bypass, OP.mult,
                    accum_out=fpart[:, COL_S1 + k:COL_S1 + k + 1])
                if k == 1:
                    tile.add_dep_helper(i_masked.ins, i_dot.ins,
                                        reason="masked after dot1")
                if k == 2:
                    tile.add_dep_helper(i_dot.ins, i_masked.ins,
                                        reason="dot2 after masked")

            # final mining pass, all on ACT:  sum_topk = sum(v*[v>t])
            # + t*(k - cnt) = sum(relu(v - t)) + t*k  exactly, so one
            # relu-accumulate is the entire tail (host adds t*k).
            nc.scalar.activation(fpart[:, COL_TCOL:COL_TCOL + 1], negt[:, :],
                                 AF.Identity, scale=-1.0)
            j4 = smallp.tile([128, G], F32, tag="sjunk")
            nc.scalar.activation(j4[:, :], pred0, AF.Identity,
                                 accum_out=fpart[:, COL_S4:COL_S4 + 1])
            rj = minep.tile([128, G], F32, tag="cjunk")
            nc.scalar.activation(rj[:, :], masked[:, :], AF.Relu,
                                 bias=negt[:, :],
                                 accum_out=fpart[:, COL_RELU:COL_RELU + 1])

            # ---- per-partition partials out; host does the final combine ----
            nc.gpsimd.dma_start(d_out[:, :], fpart[:, :])

    nc.compile()
    return nc


_nc = None


def _pad_shard(src, inner, fill):
    """[BL, P, inner] -> [BL, PP, inner] with constant-filled pad anchors."""
    out = np.empty((BL, PP, inner), np.float32)
    out[:, :P] = src
    out[:, P:] = fill
    return out


def kernel(actual_bbox_deltas, actual_labels, pred_bbox_deltas, pred_labels):
    global _nc
    if _nc is None:
        _nc = build()

    in_maps = []
    for core in range(NCORES):
        r0 = core * BL
        in_maps.append({
            "actual_bbox_deltas": _pad_shard(
                actual_bbox_deltas[r0:r0 + BL], 4, 1.0),
            "actual_labels": _pad_shard(actual_labels[r0:r0 + BL], C, 0.0),
            "pred_bbox_deltas": _pad_shard(
                pred_bbox_deltas[r0:r0 + BL], 4, 1.0),
            "pred_labels": _pad_shard(pred_labels[r0:r0 + BL], C, 0.0),
        })

    res = run_bass_kernel_spmd(_nc, in_maps, core_ids=list(range(NCORES)))
    loc = conf = pos = 0.0
    npad_core = BL * NPAD
    for core in range(NCORES):
        o = res.results[core]["out"].astype(np.float64)
        s1 = o[:, COL_S1:COL_S1 + len(LBL_CH)].sum()
        loc += o[:, COL_LOC].sum()
        neg = (o[:, COL_RELU]
               + o[:, COL_TCOL] * o[:, COL_KCOL] / PPR).sum()
        conf += (o[:, COL_S2].sum() - npad_core * LN81) - s1 \
            + o[:, COL_S4].sum() - o[:, COL_S3].sum() + neg
        pos += o[:, COL_POS].sum() - npad_core
    if pos == 0:
        return (np.float32(0.0), np.float32(0.0))
    return (np.float32(loc / pos), np.float32(conf / pos))
